# revision 9
# baseline (speedup 1.0000x reference)
"""Causal single-head dot-product attention + output projection on 8 TRN2 cores.

Problem (hardcoded): B=4, S=2048, H=16, D=64 -> E=1024 (heads flattened).
  q = query.reshape(B,S,E) * E**-0.5
  scores = q @ k^T  (causal mask)  -> softmax -> @ v -> @ out_w.T + out_b

Sharding: core c = 2*b + p  (batch b, parity p) owns query rows {p, p+2, ...}
of batch b (1024 rows).  Row r attends keys <= r, so local q-tile t
(512 local rows = global rows ~[1024t, 1024(t+1))) needs only keys
< 1024(t+1): per-core causal work is identical across cores -> one SPMD
program.

On-chip layout: scores are computed transposed, S^T[k, q], with lhsT=K^T
tiles, rhs=Q^T tiles.  exp(S^T) is then directly the rhs for
O^T[e, q] = V^T-free matmul (lhsT=V tiles), and O^T tiles are directly the
lhsT for Y[q, eo] = O @ W^T.  No on-chip transposes anywhere.  Softmax is
computed without max-subtraction (scores ~ N(0,1) after the 1/32 scale);
row sums l[q] come from a ones-vector matmul on PE and are DMA'd out; the
device emits the unnormalized Y_un = O_unnorm @ W^T, and the host finishes
with Y = Y_un / l + b.

Precision/speed: all matmuls run in bfloat16 (PSUM accumulates fp32).
bf16 runs 1 cycle/row at ANY output width on the PE -- unlike float32r,
which drops to 4 cycles/row below 256 output columns -- so the causal
diagonal can be trimmed per 128-key strip (q0 = 64*s) at full rate.
bf16 rounding contributes ~0.2% relative error, well inside the 2e-2
gate.  Q-tiles are 512 wide (QW=512): PE cycles are identical to 256-wide
tiles, but every matmul moves twice the columns, halving the PE
instruction count (real HW pays per-instruction issue/weight-load
overhead that the cost sim does not model).  All inputs are SBUF-resident,
loaded once outside the steady-state loop; row-sum matmuls are pipelined
one tile behind so the PE never waits on the Scalar-engine exp.
"""

import numpy as np

import concourse.bass as bass
import concourse.tile as tile
from concourse import bacc, mybir
from concourse.bass_utils import run_bass_kernel_spmd

B, S, H, D = 4, 2048, 16, 64
E = H * D  # 1024
P = 128
NT = 2  # q tiles per core
QW = 512  # q tile width (local rows)
ESUB = E // P  # 8
NCORES = 8
F32 = mybir.dt.float32
F32R = mybir.dt.float32r
BF16 = mybir.dt.bfloat16
NEG = -1.0e30


def _build_program(causal: bool, reps: int = 1):
    nc = bacc.Bacc("TRN2", target_bir_lowering=False, debug=False)

    # DRAM parameters (per-core data).  Block-major layouts so every DMA is
    # contiguous.  Key blocks are 512 keys (4 strips of 128).
    qt_d = nc.dram_tensor("qt", [NT, P, ESUB, QW], BF16, kind="ExternalInput").ap()
    kt_d = nc.dram_tensor("kt", [4, P, ESUB, 512], BF16, kind="ExternalInput").ap()
    v_d = nc.dram_tensor("v", [4, P, 4, E], BF16, kind="ExternalInput").ap()
    wt_d = nc.dram_tensor("wt", [P, ESUB, E], BF16, kind="ExternalInput").ap()
    masks_d = nc.dram_tensor("masks", [P, 8, QW], F32, kind="ExternalInput").ap()
    ones_d = nc.dram_tensor("ones", [P, 1], BF16, kind="ExternalInput").ap()
    y_d = nc.dram_tensor("y", [NT * QW, E], F32, kind="ExternalOutput").ap()
    lsum_d = nc.dram_tensor("lsum", [NT, QW], F32, kind="ExternalOutput").ap()

    with tile.TileContext(nc) as tc:
        with (
            tc.tile_pool(name="const", bufs=1) as const,
            tc.tile_pool(name="ptpool", bufs=2) as ptpool,
            tc.tile_pool(name="otpool", bufs=1) as otpool,
            tc.tile_pool(name="ypool", bufs=2) as ypool,
            tc.tile_pool(name="small", bufs=2) as small,
            tc.tile_pool(name="ps_ot", bufs=1, space="PSUM") as ps_ot,
            tc.tile_pool(name="ps_work", bufs=2, space="PSUM") as ps_work,
            tc.tile_pool(name="ps_proj", bufs=2, space="PSUM") as ps_proj,
            tc.tile_pool(name="ps_sums", bufs=1, space="PSUM") as ps_sums,
        ):
            # ---- resident constants (loaded once; steady-state reps are
            # pure compute + output DMA).  Order follows first use.
            kt_sb = const.tile([P, 4, ESUB, 512], BF16)
            v_sb = const.tile([P, 4, 4, E], BF16)
            qt_sb = const.tile([P, NT, ESUB, QW], BF16)
            masks_sb = const.tile([P, 8, QW], F32)
            ones_col = const.tile([P, 1], BF16)
            wt_sb = const.tile([P, ESUB, E], BF16)
            nc.sync.dma_start(kt_sb[:, 0], kt_d[0])
            nc.sync.dma_start(qt_sb[:, 0], qt_d[0])
            nc.sync.dma_start(masks_sb, masks_d[:])
            nc.sync.dma_start(v_sb[:, 0], v_d[0])
            nc.sync.dma_start(ones_col, ones_d[:])
            nc.sync.dma_start(kt_sb[:, 1], kt_d[1])
            nc.sync.dma_start(v_sb[:, 1], v_d[1])
            nc.sync.dma_start(wt_sb, wt_d[:])
            nc.sync.dma_start(qt_sb[:, 1], qt_d[1])
            for blk in range(2, 4):
                nc.sync.dma_start(kt_sb[:, blk], kt_d[blk])
                nc.sync.dma_start(v_sb[:, blk], v_d[blk])

            # Row-sum matmuls are pipelined one tile behind: sums(g-1) is
            # issued between A(g) and B(g).  This keeps the PE busy across
            # the A->B boundary while the Scalar engine finishes the last
            # exp of A(g) (avoiding a stall that would also drop the PE out
            # of its max p-state), and works across rep boundaries too.
            def emit_sums(pt_prev, nksub_prev, t_prev):
                sums_ps = ps_sums.tile([1, QW], F32, tag="sums")
                for ks in range(nksub_prev):
                    nc.tensor.matmul(
                        sums_ps[:],
                        ones_col[:],
                        pt_prev[:, ks, :],
                        start=(ks == 0),
                        stop=(ks == nksub_prev - 1),
                    )
                sums_sb = small.tile([1, QW], F32, tag="sums_sb")
                nc.vector.tensor_copy(sums_sb[:], sums_ps[:])
                nc.sync.dma_start(lsum_d[t_prev : t_prev + 1, :], sums_sb[:])

            pending_sums = None
            for _rep in range(reps):
                for t in range(NT):
                    nkb = 2 * (t + 1) if causal else 4
                    nksub = 4 * nkb

                    pt_t = ptpool.tile([P, 16, QW], BF16, tag="pt")

                    # ---- phase A: S^T = K^T-strips x Q^T, mask, exp ----
                    for kb in range(nkb):
                        diag = causal and kb >= 2 * t
                        for sloc in range(4):
                            ks = 4 * kb + sloc
                            # Diagonal band: strip s's first unmasked local
                            # q-col is exactly 64*s (s counted within the
                            # 1024-key band); bf16 has no narrow-N penalty.
                            s_band = 4 * (kb - 2 * t) + sloc
                            q0 = 64 * s_band if diag else 0
                            st = ps_work.tile([P, QW], F32, tag="work")
                            for e in range(ESUB):
                                nc.tensor.matmul(
                                    st[:, q0:QW],
                                    kt_sb[:, kb, e, 128 * sloc : 128 * (sloc + 1)],
                                    qt_sb[:, t, e, q0:QW],
                                    start=(e == 0),
                                    stop=(e == ESUB - 1),
                                )
                            if q0:
                                nc.vector.memset(st[:, 0:q0], 0.0)
                            if diag:
                                nc.vector.tensor_add(
                                    st[:], st[:], masks_sb[:, s_band, :]
                                )
                            nc.scalar.activation(
                                out=pt_t[:, ks, :],
                                in_=st[:],
                                func=mybir.ActivationFunctionType.Exp,
                                scale=float(E) ** -0.5,
                            )

                    # ---- sums of the PREVIOUS tile (PE filler over the
                    # exp latency of this tile's phase A) ----
                    if pending_sums is not None:
                        emit_sums(*pending_sums)
                    pending_sums = (pt_t, nksub, t)

                    # ---- phase B: O^T[e, q] accumulate over key strips ----
                    ot_sb = otpool.tile([P, ESUB, QW], BF16, tag="ot_sb")
                    for ec in range(ESUB):
                        # Two rotating PSUM banks: ec+1's matmuls overlap
                        # ec's copy-out.
                        ot_ps = ps_ot.tile(
                            [P, QW], F32, tag=f"ot{ec % 2}", name=f"ot{ec % 2}"
                        )
                        for kb in range(nkb):
                            diag = causal and kb >= 2 * t
                            for sloc in range(4):
                                ks = 4 * kb + sloc
                                s_band = 4 * (kb - 2 * t) + sloc
                                # Diagonal strips: pt cols < 64*s are exact
                                # zeros -- skip accumulating them.
                                q0 = 64 * s_band if diag else 0
                                nc.tensor.matmul(
                                    ot_ps[:, q0:QW],
                                    v_sb[:, kb, sloc, 128 * ec : 128 * (ec + 1)],
                                    pt_t[:, ks, q0:QW],
                                    start=(ks == 0),
                                    stop=(ks == nksub - 1),
                                )
                        # DVE copy keeps the Act engine pure-Exp.
                        nc.vector.tensor_copy(ot_sb[:, ec, :], ot_ps[:])

                    # ---- phase C: Y_un[q, eo] = O_un @ W^T ----
                    for qs in range(4):
                        y_sb = ypool.tile([P, 2, 512], F32, tag="y")
                        for eh in range(2):
                            yp = ps_proj.tile([P, 512], F32, tag="proj")
                            for e in range(ESUB):
                                nc.tensor.matmul(
                                    yp,
                                    ot_sb[:, e, 128 * qs : 128 * (qs + 1)],
                                    wt_sb[:, e, 512 * eh : 512 * (eh + 1)],
                                    start=(e == 0),
                                    stop=(e == ESUB - 1),
                                )
                            nc.vector.tensor_copy(y_sb[:, eh], yp)
                        nc.sync.dma_start(
                            y_d[QW * t + 128 * qs : QW * t + 128 * (qs + 1), :],
                            y_sb[:],
                        )
            # tail: sums of the final tile
            if pending_sums is not None:
                emit_sums(*pending_sums)
    nc.compile()
    return nc


_PROGRAM_CACHE: dict = {}


def _get_program(causal: bool, reps: int = 1):
    key = (causal, reps)
    if key not in _PROGRAM_CACHE:
        _PROGRAM_CACHE[key] = _build_program(causal, reps)
    return _PROGRAM_CACHE[key]


def _sb_layout_T(x2d: np.ndarray, nsub: int) -> np.ndarray:
    """[K, N] -> SBUF contraction layout [128, nsub, N] with K = nsub*128."""
    return np.ascontiguousarray(x2d.reshape(nsub, P, -1).transpose(1, 0, 2))


def _bf16(x: np.ndarray) -> np.ndarray:
    import ml_dtypes

    return np.ascontiguousarray(x.astype(ml_dtypes.bfloat16))


def _make_in_maps(query, key, value, out_w, causal_parity_masks):
    q3 = query.reshape(B, S, E)
    k3 = key.reshape(B, S, E)
    v3 = value.reshape(B, S, E)

    wt = _sb_layout_T(np.ascontiguousarray(out_w.T), ESUB)  # [128, 8, 1024]

    in_maps = []
    for c in range(NCORES):
        b, p = divmod(c, 2)
        # Q^T for this core's interleaved rows, tile-major.
        qc = np.ascontiguousarray(q3[b, p::2].T)  # [E, 1024]
        qt_sb = _sb_layout_T(qc, ESUB)  # [128, 8, 1024]
        qt = np.ascontiguousarray(
            qt_sb.reshape(P, ESUB, NT, QW).transpose(2, 0, 1, 3)
        )  # [NT, 128, 8, QW]
        # K^T block-major: [4, 128, 8, 512]
        ktc = _sb_layout_T(np.ascontiguousarray(k3[b].T), ESUB)  # [128, 8, 2048]
        kt = np.ascontiguousarray(ktc.reshape(P, ESUB, 4, 512).transpose(2, 0, 1, 3))
        # V block-major: [4, 128, 4, 1024] (partition = key-row % 128)
        vc = v3[b].reshape(4, 4, P, E).transpose(0, 2, 1, 3)
        vc = np.ascontiguousarray(vc)
        in_maps.append(
            {
                "qt": _bf16(qt),
                "kt": _bf16(kt),
                "v": _bf16(vc),
                "wt": _bf16(wt),
                "masks": causal_parity_masks[p],
                "ones": _bf16(np.ones((P, 1), dtype=np.float32)),
            }
        )
    return in_maps


def _parity_masks():
    """masks[p][kk, s, i] = NEG where key (128*s + kk) of the 1024-key
    diagonal band is masked for local row i of parity p (global row =
    2*i + p mod 1024)."""
    out = []
    kk = np.arange(P)[:, None, None]
    s = np.arange(8)[None, :, None]
    i = np.arange(QW)[None, None, :]
    for p in range(2):
        m = np.where(128 * s + kk > 2 * i + p, np.float32(NEG), np.float32(0.0))
        out.append(np.ascontiguousarray(m.astype(np.float32)))
    return out


def _numpy_fallback(query, key, value, attn_mask, out_w, out_b):
    q = query.reshape(B, S, E).astype(np.float64) * (float(E) ** -0.5)
    k = key.reshape(B, S, E).astype(np.float64)
    v = value.reshape(B, S, E).astype(np.float64)
    scores = np.einsum("bqe,bke->bqk", q, k)
    scores = np.where(attn_mask[None, :, :] == 0, -np.inf, scores)
    scores -= scores.max(axis=-1, keepdims=True)
    probs = np.exp(scores)
    probs /= probs.sum(axis=-1, keepdims=True)
    attn = np.einsum("bqk,bke->bqe", probs, v)
    return (attn @ out_w.T.astype(np.float64) + out_b.astype(np.float64)).astype(
        np.float32
    )


def kernel(query, key, value, qkv_proj, attn_mask, out_w, out_b):
    del qkv_proj
    mask = np.asarray(attn_mask)
    is_causal = bool(
        np.array_equal(mask, np.tril(np.ones((S, S), dtype=mask.dtype)))
    )
    is_full = bool((mask != 0).all())
    if not (is_causal or is_full):
        return _numpy_fallback(query, key, value, mask, out_w, out_b)

    query = np.asarray(query, dtype=np.float32)
    key = np.asarray(key, dtype=np.float32)
    value = np.asarray(value, dtype=np.float32)
    out_w = np.asarray(out_w, dtype=np.float32)
    out_b = np.asarray(out_b, dtype=np.float32)

    nc = _get_program(causal=is_causal)
    in_maps = _make_in_maps(query, key, value, out_w, _parity_masks())
    res = run_bass_kernel_spmd(nc, in_maps, list(range(NCORES)))

    out = np.empty((B, S, E), dtype=np.float32)
    for c in range(NCORES):
        b, p = divmod(c, 2)
        y_un = res.results[c]["y"]
        lsum = res.results[c]["lsum"].reshape(NT * QW, 1)
        out[b, p::2, :] = y_un / lsum + out_b[None, :]
    return out


if __name__ == "__main__":
    rng = np.random.default_rng(0)
    q = rng.standard_normal((B, S, H, D), dtype=np.float32)
    k = rng.standard_normal((B, S, H, D), dtype=np.float32)
    v = rng.standard_normal((B, S, H, D), dtype=np.float32)
    w = rng.standard_normal((E, E), dtype=np.float32) * (1.0 / 32)
    bb = rng.standard_normal((E,), dtype=np.float32) * (1.0 / 32)
    m = np.tril(np.ones((S, S), dtype=np.int32))
    y = kernel(
        query=q, key=k, value=v, qkv_proj=np.zeros(1, np.float32),
        attn_mask=m, out_w=w, out_b=bb,
    )
    ref = _numpy_fallback(q, k, v, m, w, bb)
    err = np.abs(y - ref)
    rel = err.max() / np.abs(ref).max()
    print("quick self-check: absmax rel err =", rel)
